# revision 37
# baseline (speedup 1.0000x reference)
"""Trainium2 Bass kernel for nn_MultiHeadLocalAttention (band-limited attention).

Math: scores are multiplied by a band-count matrix C that is zero outside
|q-k|<=4, then a FULL-row softmax is applied.  Out-of-band entries contribute
exp(0)=1.  For each 128-query tile j the kernel computes E = exp(C*S) over a
fixed 136-wide key strip (128 + 8 tail); strip entries with C=0 give exp(0)=1,
so with a per-tile constant correction:

  numer[q] = sum_strip E[k,q] vh[k] + (Vsum - Strip_j)
  denom[q] = sum_strip E[k,q] + (2048 - 136)

where Strip_j = sum of vh over the 136 strip rows (host precomputed, exact to
fp rounding).  This makes the O(seq^2) attention an O(seq*band) computation.

All matmuls run in bf16 (4x PE throughput vs fp32, fp32 PSUM accumulate);
measured end-to-end relative error ~7e-3 vs the 2e-2 budget.  Projections are
pair-packed (two 64-wide heads per one 128-wide stationary matmul); the psum
is then copied into head-separated base-0 slices because matmul operands at
partition base 64 (auto-derived row-group tiling) crash the HW toolchain.
The attention is software-pipelined: scores/CS-multiply/exp of tile j+1 run
on PE/DVE/ACT while numer/divide/transpose/out-projection of tile j execute.

Sharding: batch*seq rows split across 8 cores (512 rows each); each core
computes all 8 heads for its rows.  k/v inputs carry a +-4 halo.  No
collectives; one SPMD Bass/Tile program.
"""

import math
import sys
from contextlib import ExitStack

import numpy as np

sys.path.insert(0, "/opt/trn_rl_repo")

import ml_dtypes

import concourse.bass as bass
import concourse.tile as tile
from concourse import bacc, mybir
from concourse.bass_utils import run_bass_kernel_spmd

F32 = mybir.dt.float32
BF16 = mybir.dt.bfloat16
NPBF = ml_dtypes.bfloat16
SEQ, DM, H, DK = 2048, 512, 8, 64
ROWS = 512          # query rows per core
HALO = ROWS + 8     # padded k/v halo columns per core
J = 4               # 128-row query tiles per core
STRIP = 136         # key strip width per query tile (128 + 8 tail)
WBAND = 2


# ----------------------------------------------------------------------------
# host-side helpers
# ----------------------------------------------------------------------------

def _band_count(seq=SEQ, window=WBAND):
    i = np.arange(seq)
    lo = np.clip(i - window, 0, None)
    hi = np.clip(i + window, None, seq - 1)
    lo = np.where(i == 1, 0, lo)
    hi = np.where(i == 1, window + 1, hi)
    lo = np.where(i == seq - 2, seq - window - 2, lo)
    hi = np.where(i == seq - 2, seq - 1, hi)
    a = np.arange(seq)[None, :]
    M = ((a >= lo[:, None]) & (a <= hi[:, None])).astype(np.float32)
    return M.T @ M


def _c_tiles(R0, C):
    """CA [J,128,128], CB [J,8,128] with C[k,q]/sqrt(dk) (C symmetric)."""
    CA = np.zeros((J, 128, 128), np.float32)
    CB = np.zeros((J, 8, 128), np.float32)
    for j in range(J):
        qg = R0 + 128 * j + np.arange(128)
        kgA = R0 - 4 + 128 * j + np.arange(128)
        kgB = R0 - 4 + 128 * j + 128 + np.arange(8)
        mA = (kgA >= 0) & (kgA < SEQ)
        mB = (kgB >= 0) & (kgB < SEQ)
        CA[j][mA, :] = C[np.ix_(kgA[mA], qg)]
        CB[j][mB, :] = C[np.ix_(kgB[mB], qg)]
    return CA, CB


# ----------------------------------------------------------------------------
# device program
# ----------------------------------------------------------------------------

def _build_program(with_bias, safe_scores=True):
    nc = bacc.Bacc()
    E_IN = dict(kind="ExternalInput")
    CA_d = nc.dram_tensor("CA", [J, 128, 128], F32, **E_IN)
    CBs_d = nc.dram_tensor("CBs", [J, 8, 8], F32, **E_IN)
    vsj_d = nc.dram_tensor("vsj", [J, H, 65], BF16, **E_IN)
    ident_d = nc.dram_tensor("ident", [128, 128], BF16, **E_IN)
    qT_d = nc.dram_tensor("qT", [DM, ROWS], BF16, **E_IN)
    kT_d = nc.dram_tensor("kT", [DM, HALO], BF16, **E_IN)
    vT_d = nc.dram_tensor("vT", [DM, HALO], BF16, **E_IN)
    Wq_d = nc.dram_tensor("Wq", [DM, DM], BF16, **E_IN)
    Wk_d = nc.dram_tensor("Wk", [DM, DM], BF16, **E_IN)
    Wv_d = nc.dram_tensor("Wv", [DM, DM], BF16, **E_IN)
    Wo_d = nc.dram_tensor("Wo", [DM, DM], BF16, **E_IN)
    if with_bias:
        bq_d = nc.dram_tensor("bq", [1, DM], BF16, **E_IN)
        bk_d = nc.dram_tensor("bk", [1, DM], BF16, **E_IN)
        bv_d = nc.dram_tensor("bv", [1, DM], BF16, **E_IN)
    out_d = nc.dram_tensor("out", [ROWS, DM], BF16, kind="ExternalOutput")

    MULT = mybir.AluOpType.mult
    EXP = mybir.ActivationFunctionType.Exp

    with tile.TileContext(nc) as tc, ExitStack() as ctx:
        sing = ctx.enter_context(tc.tile_pool(name="sing", bufs=1))
        att = ctx.enter_context(tc.tile_pool(name="att", bufs=2))

        # --- loads: one queue so transfer order == priority order.  Wq/qT
        # gate the first projection and are chunked so the first half can
        # start compute while the rest streams. ---
        sb_Wq = sing.tile([128, 4, DM], BF16)
        wq_r = Wq_d[:].rearrange("(kc p) n -> p kc n", p=128)
        nc.sync.dma_start(sb_Wq[:, 0:2, :], wq_r[:, 0:2, :])
        sb_qT = sing.tile([128, 4, ROWS], BF16)
        qt_r = qT_d[:].rearrange("(kc p) r -> p kc r", p=128)
        nc.sync.dma_start(sb_qT[:, 0:2, :], qt_r[:, 0:2, :])
        nc.sync.dma_start(sb_Wq[:, 2:4, :], wq_r[:, 2:4, :])
        nc.sync.dma_start(sb_qT[:, 2:4, :], qt_r[:, 2:4, :])
        sb_Wk = sing.tile([128, 4, DM], BF16)
        nc.sync.dma_start(sb_Wk, Wk_d[:].rearrange("(kc p) n -> p kc n", p=128))
        sb_kT = sing.tile([128, 4, HALO], BF16)
        nc.sync.dma_start(sb_kT, kT_d[:].rearrange("(kc p) r -> p kc r", p=128))
        sb_vT = sing.tile([128, 4, HALO], BF16)
        nc.sync.dma_start(sb_vT, vT_d[:].rearrange("(kc p) r -> p kc r", p=128))
        sb_Wv = sing.tile([128, 4, DM], BF16)
        nc.sync.dma_start(sb_Wv, Wv_d[:].rearrange("(kc p) n -> p kc n", p=128))
        sb_CA = sing.tile([128, J, 128], F32)
        nc.sync.dma_start(sb_CA, CA_d[:].rearrange("j p q -> p j q"))
        sb_CBs = sing.tile([8, J, 8], F32)
        nc.sync.dma_start(sb_CBs, CBs_d[:].rearrange("j p q -> p j q"))
        sb_id = sing.tile([128, 128], BF16)
        nc.sync.dma_start(sb_id, ident_d[:])
        # vnext[0:8] = vh rows of the next 128-block; vnext[8] = strip
        # correction row (Vsum - Strip_j | 1912), paired with eB row 8 = 1.
        sb_vnext = sing.tile([9, J, H, 65], BF16)
        nc.sync.dma_start(sb_vnext[8:9, :, :, :],
                          vsj_d[:].rearrange("j h c -> (j h) c"))
        sb_Wo = sing.tile([128, 4, DM], BF16)
        nc.sync.dma_start(sb_Wo, Wo_d[:].rearrange("(kc p) n -> p kc n", p=128))
        if with_bias:
            sb_bq = sing.tile([1, DM], BF16)
            nc.sync.dma_start(sb_bq, bq_d[:])
            sb_bk = sing.tile([1, DM], BF16)
            nc.sync.dma_start(sb_bk, bk_d[:])
            sb_bv = sing.tile([1, DM], BF16)
            nc.sync.dma_start(sb_bv, bv_d[:])

        sb_ones_r = sing.tile([1, ROWS], BF16)    # ones row (warmup/bias rhs)
        nc.vector.memset(sb_ones_r, 1.0)
        # exp(C*S) piece B on rows 0:8; row 8 stays 1.0 and pairs with the
        # vnext correction row so one K=9 matmul covers tail + correction.
        sb_eB = sing.tile([9, H * 128], BF16)
        nc.vector.memset(sb_eB, 1.0)

        # persistent intermediates (head-separated at partition base 0)
        sb_qhT = sing.tile([64, H, ROWS], BF16)
        sb_khT = sing.tile([64, H, HALO], BF16)

        def qh_sl(h, lo, hi):
            return sb_qhT[:, h, lo:hi]

        def kh_sl(h, lo, hi):
            return sb_khT[:, h, lo:hi]

        sb_vh = sing.tile([128, 5, H, 65], BF16)  # shifted row tiles, 65 = 64d+1
        nc.vector.memset(sb_vh[:, :, :, 64:65], 1.0)
        sb_concat = sing.tile([128, J, DM], BF16)
        sb_concatT = sing.tile([128, 4, ROWS], BF16)

        copy_engines = [nc.scalar.copy, nc.vector.tensor_copy]

        def cpy(i, dst, src):
            copy_engines[i % 2](dst, src)

        psa_t = [None] * J
        eA_t = [None] * J
        pn_t = [None] * J

        def scores_a(pool, j, bufs, fillers=0):
            psa = pool.tile([128, H * 128], F32, tag="sa", bufs=bufs,
                            name=f"psa{j}")
            psa_t[j] = psa
            # dummy matmuls keep the PE clock ramped across upstream waits;
            # the h=0 start=True matmul below overwrites their output
            for _ in range(fillers):
                nc.tensor.matmul(psa[:, 0:128], sb_ones_r[0:1, 0:128],
                                 sb_ones_r[0:1, 0:128], start=True, stop=True,
                                 skip_group_check=True)
            for h in range(H):
                nc.tensor.matmul(
                    psa[:, 128 * h:128 * h + 128],
                    kh_sl(h, 128 * j, 128 * j + 128),
                    qh_sl(h, 128 * j, 128 * j + 128),
                    start=True, stop=True)

        def cs_exp(j):
            psa = psa_t[j]
            ca = sb_CA[:, j, :]
            ca_b = bass.AP(tensor=ca.tensor, offset=ca.offset,
                           ap=[list(ca.ap[0]), [0, H], list(ca.ap[1])])
            psa_v = psa[:].rearrange("p (h q) -> p h q", h=H)
            nc.vector.tensor_mul(psa_v, psa_v, ca_b)
            eA = att.tile([128, H * 128], BF16, tag="eA", name=f"eA{j}")
            eA_t[j] = eA
            nc.scalar.activation(eA, psa, EXP)

        def scores_b(pool, j):
            pn0 = pool.tile([128, 4, 65], F32, tag="n", bufs=2,
                            name=f"pn0_{j}")
            pn_t[j] = pn0
            psb = pn0[0:8, 0, 0:64].rearrange("p (h q) -> p h q", h=H)
            for h in range(H):
                nc.tensor.matmul(
                    psb[:, h, :],
                    kh_sl(h, 128 * j + 128, 128 * j + 136),
                    qh_sl(h, 128 * j + 120, 128 * j + 128),
                    start=True, stop=True, skip_group_check=True)
            cb = sb_CBs[:, j, :]
            cb_b = bass.AP(tensor=cb.tensor, offset=cb.offset,
                           ap=[list(cb.ap[0]), [0, H], list(cb.ap[1])])
            nc.vector.tensor_mul(psb, psb, cb_b)
            eB_v = sb_eB[0:8, :].rearrange("p (h q) -> p h q",
                                           h=H)[:, :, 120:128]
            nc.scalar.activation(eB_v, psb, EXP)

        # ---------------- phase 1: projections ----------------
        # PSUM: pj 2 + pt 1 + pv 2 + sa 2 = 7 banks while ph1 is open.
        with tc.tile_pool(name="ph1", bufs=2, space="PSUM") as ph1:
            # qh/kh pair-packed matmuls: psum [128, 512] covers 2 heads
            for hp in range(4):
                ps = ph1.tile([128, ROWS], F32, tag="pj")
                for kc in range(4):
                    nc.tensor.matmul(ps, sb_Wq[:, kc, 128 * hp:128 * hp + 128],
                                     sb_qT[:, kc, :], start=(kc == 0),
                                     stop=(kc == 3 and not with_bias))
                if with_bias:
                    nc.tensor.matmul(ps, sb_bq[0:1, 128 * hp:128 * hp + 128],
                                     sb_ones_r, start=False, stop=True)
                cpy(hp, sb_qhT[:, 2 * hp, :], ps[0:64, :])
                cpy(hp + 1, sb_qhT[:, 2 * hp + 1, :], ps[64:128, :])
            for hp in range(4):
                ps = ph1.tile([128, ROWS], F32, tag="pj")
                pst = ph1.tile([128, 8], F32, tag="pt", bufs=1)
                for kc in range(4):
                    nc.tensor.matmul(ps, sb_Wk[:, kc, 128 * hp:128 * hp + 128],
                                     sb_kT[:, kc, 0:512], start=(kc == 0),
                                     stop=(kc == 3 and not with_bias))
                    nc.tensor.matmul(pst, sb_Wk[:, kc, 128 * hp:128 * hp + 128],
                                     sb_kT[:, kc, 512:HALO], start=(kc == 0),
                                     stop=(kc == 3 and not with_bias))
                if with_bias:
                    nc.tensor.matmul(ps, sb_bk[0:1, 128 * hp:128 * hp + 128],
                                     sb_ones_r, start=False, stop=True)
                    nc.tensor.matmul(pst, sb_bk[0:1, 128 * hp:128 * hp + 128],
                                     sb_ones_r[0:1, 0:8], start=False, stop=True)
                cpy(hp, sb_khT[:, 2 * hp, 0:512], ps[0:64, :])
                cpy(hp + 1, sb_khT[:, 2 * hp + 1, 0:512], ps[64:128, :])
                cpy(hp, sb_khT[:, 2 * hp, 512:HALO], pst[0:64, :])
                cpy(hp + 1, sb_khT[:, 2 * hp + 1, 512:HALO], pst[64:128, :])
            # tile-0 scores + exp start here so the DVE/ACT chain overlaps
            # the vh projections below
            scores_a(ph1, 0, 1)
            cs_exp(0)
            # vh[rows(+halo shift), dout] in 65-strided head blocks
            for rt in range(5):
                nr = 128 if rt < 4 else 8
                ps = ph1.tile([128, DM], F32, tag="pv")
                for kc in range(4):
                    nc.tensor.matmul(ps[0:nr, :],
                                     sb_vT[:, kc, 128 * rt:128 * rt + nr],
                                     sb_Wv[:, kc, :], start=(kc == 0),
                                     stop=(kc == 3 and not with_bias))
                if with_bias:
                    nc.tensor.matmul(ps[0:nr, :], sb_ones_r[0:1, 0:nr], sb_bv,
                                     start=False, stop=True)
                cpy(rt, sb_vh[0:nr, rt, :, 0:64],
                    ps[0:nr, :].rearrange("p (h d) -> p h d", h=H))
                if rt >= 1:
                    nc.gpsimd.tensor_copy(sb_vnext[0:8, rt - 1, :, :],
                                          sb_vh[0:8, rt, :, :])

        # ------- phase 2: per-tile attention + transpose + out projection ----
        # Software-pipelined: scores/CS/exp for tile j+1 run on PE/DVE/ACT
        # while numer/divide/transpose/out-proj of tile j execute, so the
        # exp chain never stalls the in-order PE queue.
        # PSUM: sa 4 + n 2 + t 1 + f 1 = 8 banks.
        with tc.tile_pool(name="pA", bufs=1, space="PSUM") as pA:
            scores_b(pA, 0)
            scores_a(pA, 1, 2)
            for j in range(J):
                if j + 1 < J:
                    cs_exp(j + 1)
                # numer: [128, 65] per head, 4 heads per psum tile;
                # eB row 8 (ones) times vnext row 8 adds the correction
                eA = eA_t[j]
                pn0 = pn_t[j]
                pn1 = pA.tile([128, 4, 65], F32, tag="n", bufs=2,
                              name=f"pn1_{j}")
                for h in range(H):
                    pn = (pn0 if h < 4 else pn1)[:, h % 4, :]
                    nc.tensor.matmul(pn, eA[:, 128 * h:128 * h + 128],
                                     sb_vh[:, j, h, :], start=True, stop=False,
                                     skip_group_check=True)
                    nc.tensor.matmul(pn, sb_eB[:, 128 * h:128 * h + 128],
                                     sb_vnext[:, j, h, :], start=False,
                                     stop=True, skip_group_check=True)
                # divide + write concat
                r = att.tile([128, H], F32, tag="r")
                nc.vector.reciprocal(r[:, 0:4], pn0[:, :, 64])
                nc.vector.reciprocal(r[:, 4:8], pn1[:, :, 64])
                for pn, ho in ((pn0, 0), (pn1, 4)):
                    rs = r[:, ho:ho + 4]
                    r_b = bass.AP(tensor=rs.tensor, offset=rs.offset,
                                  ap=[list(rs.ap[0]), list(rs.ap[1]), [0, 64]])
                    outv = sb_concat[:, j, 64 * ho:64 * ho + 256]
                    nc.vector.scalar_tensor_tensor(
                        outv.rearrange("p (h d) -> p h d", h=4),
                        pn[:, :, 0:64], 1.0, r_b, op0=MULT, op1=MULT)
                # next tile's tail scores reuse pn0's bank after the divide
                if j + 1 < J:
                    scores_b(pA, j + 1)
                # transpose tile j: concat [128q, 512d] -> concatT[:, :, qj]
                pt = pA.tile([128, 4, 128], BF16, tag="t", bufs=1)
                for dc in range(4):
                    nc.tensor.transpose(pt[:, dc, :],
                                        sb_concat[:, j, 128 * dc:128 * dc + 128],
                                        sb_id)
                nc.scalar.copy(sb_concatT[:, :, 128 * j:128 * j + 128], pt)
                # output projection for query rows of tile j
                pf = pA.tile([128, DM], F32, tag="f", bufs=1)
                for dc in range(4):
                    nc.tensor.matmul(pf, sb_concatT[:, dc, 128 * j:128 * j + 128],
                                     sb_Wo[:, dc, :], start=(dc == 0),
                                     stop=(dc == 3))
                so = att.tile([128, DM], BF16, tag="so")
                nc.scalar.copy(so, pf)
                nc.sync.dma_start(out_d[128 * j:128 * j + 128, :], so)
                # queue the next-next tile's scores behind this tile's work
                if j + 2 < J:
                    scores_a(pA, j + 2, 2)

    if not nc.is_finalized():
        nc.finalize()
    return nc


_PROG_CACHE = {}


def _get_program(with_bias):
    key = bool(with_bias)
    if key not in _PROG_CACHE:
        _PROG_CACHE[key] = _build_program(key)
    return _PROG_CACHE[key]


# ----------------------------------------------------------------------------
# entry point
# ----------------------------------------------------------------------------

def prep_in_maps(q, k, v, Wq, bq, Wk, bk, Wv, bv, Wo, bo, **_unused):
    """Builds per-core input maps + the traced program; returns (in_maps, nc)."""
    q = np.asarray(q, np.float32)
    k = np.asarray(k, np.float32)
    v = np.asarray(v, np.float32)
    Wq = np.ascontiguousarray(Wq, np.float32)
    Wk = np.ascontiguousarray(Wk, np.float32)
    Wv = np.ascontiguousarray(Wv, np.float32)
    Wo = np.ascontiguousarray(Wo, np.float32)
    bq = np.asarray(bq, np.float32).reshape(-1)
    bk = np.asarray(bk, np.float32).reshape(-1)
    bv = np.asarray(bv, np.float32).reshape(-1)
    with_bias = bool(np.any(bq) or np.any(bk) or np.any(bv))
    nc = _get_program(with_bias)

    C = _band_count() / np.float32(math.sqrt(DK))
    ident = np.eye(128, dtype=NPBF)
    Wq_b, Wk_b = Wq.astype(NPBF), Wk.astype(NPBF)
    Wv_b, Wo_b = Wv.astype(NPBF), Wo.astype(NPBF)
    q_b = q.astype(NPBF)
    vfull = v.sum(axis=1)  # [2, 512]

    in_maps = []
    for c in range(8):
        b, R0 = c // 4, ROWS * (c % 4)
        qT = np.ascontiguousarray(q_b[b, R0:R0 + ROWS, :].T)
        kT = np.zeros((DM, HALO), NPBF)
        vT = np.zeros((DM, HALO), NPBF)
        g0 = R0 - 4
        s0, s1 = max(g0, 0), min(R0 + ROWS + 4, SEQ)
        kT[:, s0 - g0:s1 - g0] = k[b, s0:s1, :].astype(NPBF).T
        vT[:, s0 - g0:s1 - g0] = v[b, s0:s1, :].astype(NPBF).T
        CA, CB = _c_tiles(R0, C)
        CBs = np.ascontiguousarray(CB[:, :, 120:128])
        assert not CB[:, :, :120].any()
        # per-tile strip corrections: (Vsum - Strip_j) @ Wv, count 2048-136
        vsj = np.empty((J, H, 65), np.float32)
        for j in range(J):
            t0, t1 = max(R0 + 128 * j - 4, 0), min(R0 + 128 * j + 132, SEQ)
            strip = v[b, t0:t1, :].sum(axis=0)
            corr = (vfull[b] - strip) @ Wv
            if with_bias:
                corr += (SEQ - STRIP) * bv
            vsj[j, :, :64] = corr.reshape(H, DK)
            vsj[j, :, 64] = float(SEQ - STRIP)
        m = {"qT": qT, "kT": kT, "vT": vT, "Wq": Wq_b, "Wk": Wk_b, "Wv": Wv_b,
             "Wo": Wo_b, "CA": CA, "CBs": CBs, "vsj": vsj.astype(NPBF),
             "ident": ident}
        if with_bias:
            m["bq"] = bq[None, :].astype(NPBF)
            m["bk"] = bk[None, :].astype(NPBF)
            m["bv"] = bv[None, :].astype(NPBF)
        in_maps.append(m)
    return in_maps, nc


def kernel(q, k, v, Wq, bq, Wk, bk, Wv, bv, Wo, bo, **_unused):
    bo = np.asarray(bo, np.float32).reshape(-1)
    in_maps, nc = prep_in_maps(q, k, v, Wq, bq, Wk, bk, Wv, bv, Wo, bo)
    res = run_bass_kernel_spmd(nc, in_maps, core_ids=list(range(8)))
    out = np.empty((2, SEQ, DM), np.float32)
    for c in range(8):
        b, R0 = c // 4, ROWS * (c % 4)
        out[b, R0:R0 + ROWS, :] = res.results[c]["out"].astype(np.float32)
    if np.any(bo):
        out += bo
    return out


if __name__ == "__main__":
    rng = np.random.default_rng(0)
    s = 1.0 / math.sqrt(DM)
    inp = dict(
        q=rng.standard_normal((2, SEQ, DM)).astype(np.float32),
        k=rng.standard_normal((2, SEQ, DM)).astype(np.float32),
        v=rng.standard_normal((2, SEQ, DM)).astype(np.float32),
        Wq=(rng.standard_normal((DM, DM)) * s).astype(np.float32),
        bq=np.zeros(DM, np.float32),
        Wk=(rng.standard_normal((DM, DM)) * s).astype(np.float32),
        bk=np.zeros(DM, np.float32),
        Wv=(rng.standard_normal((DM, DM)) * s).astype(np.float32),
        bv=np.zeros(DM, np.float32),
        Wo=(rng.standard_normal((DM, DM)) * s).astype(np.float32),
        bo=np.zeros(DM, np.float32),
    )
    out = kernel(**inp)
    print("kernel ran, out shape", out.shape, "mean", np.abs(out).mean())
